# revision 1
# baseline (speedup 1.0000x reference)
"""Trainium2 Bass kernel: masked multi-head attention, sharded across 8 NeuronCores.

Problem shapes (hardcoded): B=2, T=2048, D=1024, H=16 heads, dh=64.

Sharding: one SPMD program with two phases (one per batch element). In each
phase every core handles 2 of the 16 heads (core c -> heads 2c, 2c+1), so the
16 heads of each batch are spread over all 8 cores. This load-balances the
data-dependent work (Q_len/V_len trim the q/k tile counts per batch).

Device algorithm per phase, per core:
  - project kT [128=2*64, Kp] and qT [128, Qp] (heads stacked on partition
    halves), and v_aug [128, NK, 2, 65] (natural token-major layout with a
    ones-column at index 64 per head, so the PV matmul's psum row 64 is the
    softmax denominator)
  - per 512-wide q chunk, per 128-wide key tile kt:
      S^T[kt] = kT_tile.T @ qT_chunk              (PE, K=64, heads row-packed)
      E = exp(scale*S^T + kbias)                  (ACT; kbias masks padded keys)
      [d; O^T*d] += v_aug.T @ E                   (PE, K=128; row 0 = sum = d)
  - epilogue: r = qmask / d (DVE), broadcast r over partitions with a K=1
    ones matmul (PE), O^T_normalized = O^T * r (DVE), DMA out.
Host transposes/pads inputs into DMA-friendly layouts and transposes the
per-core [64, Qp] head outputs back into the [B, T, 1024] result.
"""

import math
import os
from contextlib import ExitStack

import numpy as np

import concourse.bacc as bacc
import concourse.mybir as mybir
import concourse.tile as tile
from concourse.bass_utils import run_bass_kernel_spmd

F32 = mybir.dt.float32
F16 = mybir.dt.float16
EXP = mybir.ActivationFunctionType.Exp
USE_FP16 = os.environ.get("MHA_FP16_INPUTS", "") == "1"
XDT = F16 if USE_FP16 else F32
XNP = np.float16 if USE_FP16 else np.float32

B, T, D, H, DH = 2, 2048, 1024, 16, 64
N_CORES = 8
KCH = D // 128          # 8 contraction chunks of the model dim
NEG_BIG = 1.0e12
SCALE = 1.0 / math.sqrt(DH)

LAST_EXEC_NS = None     # filled when BASS_TRACE=1


def _ensure_ntff_hook():
    """run_bass_kernel_spmd(trace=True) imports antenv.axon_hooks, which some
    containers lack; synthesize it (backed by libaxon_pjrt's NRT profiling)
    so tracing degrades gracefully instead of crashing."""
    import sys
    import types
    try:
        import antenv.axon_hooks  # noqa: F401
        return
    except ImportError:
        pass
    try:
        import antenv
        from trn_agent_boot.trn_boot import _ntff_profile_via_ctypes
        hook = _ntff_profile_via_ctypes("/opt/axon/libaxon_pjrt.so")
    except Exception:
        antenv = None
        hook = None
    try:
        m = types.ModuleType("antenv.axon_hooks")
        m._hook = hook
        m.set_axon_ntff_profile_hook = lambda h: setattr(m, "_hook", h)
        m.get_axon_ntff_profile_hook = lambda: m._hook
        sys.modules["antenv.axon_hooks"] = m
        if antenv is not None:
            antenv.axon_hooks = m
    except Exception:
        pass


def _ceil_div(a, b):
    return -(-a // b)


def _emit_phase(nc, tc, P, ph):
    """Emit one batch element's phase into the program."""
    s = str(ph["b"])
    io = ph["io"]
    NQ, NK, Qp, Kp = ph["NQ"], ph["NK"], ph["Qp"], ph["Kp"]
    scale = ph["scale"]

    # --- constants / masks (weights are shared across phases) ---
    wts = P["wts"]
    kb = P["w"].tile([128, NK], F32, tag="kb" + s, name="kb" + s)
    nc.sync.dma_start(kb[:], io["kb"][:])
    qm = P["w"].tile([65, Qp], F32, tag="qm" + s, name="qm" + s)
    nc.sync.dma_start(qm[64:65, :], io["qm"][:])

    # --- k projection: kT chunks [128(outd for 2 heads), <=512 keys] ---
    kcs = []
    for c in range(_ceil_div(Kp, 512)):
        n = min(512, Kp - c * 512)
        xt = P["x"].tile([128, KCH, n], XDT, tag="xt", name="xt")
        if ph.get("first") and c == 0:
            # per-k-slice DMAs let the first projection matmul start as
            # soon as slice 0 lands instead of after the whole 2MB chunk
            for k in range(KCH):
                nc.sync.dma_start(xt[:, k, :], io["xk"][:, k, c * 512:c * 512 + n])
        else:
            nc.sync.dma_start(xt[:], io["xk"][:, :, c * 512:c * 512 + n])
        ps = P["pp"].tile([128, n], F32, tag="pp", name="pp")
        for k in range(KCH):
            nc.tensor.matmul(ps[:], lhsT=wts["wk"][:, k, :], rhs=xt[:, k, :],
                             start=(k == 0), stop=(k == KCH - 1))
        kc = P["persist"].tile([128, n], F32, tag="kT" + s, name="kT" + s,
                               bufs=_ceil_div(Kp, 512))
        nc.vector.tensor_copy(kc[:], ps[:])
        kcs.append(kc)

    # --- v projection into v_aug tiles [128 tokens, 2 heads, 1+64] ---
    vas = []
    for c in range(_ceil_div(Kp, 512)):
        n = min(512, Kp - c * 512)
        xt = P["x"].tile([128, KCH, n], XDT, tag="xt", name="xt")
        nc.sync.dma_start(xt[:], io["xv"][:, :, c * 512:c * 512 + n])
        for m in range(n // 128):
            va = P["persist"].tile([128, 2, 65], F32, tag="va" + s, name="va" + s,
                                   bufs=NK)
            nc.vector.memset(va[:, :, 64:65], 1.0)
            ps = P["pp"].tile([128, 128], F32, tag="pp", name="pp")
            for k in range(KCH):
                nc.tensor.matmul(ps[:], lhsT=xt[:, k, m * 128:(m + 1) * 128],
                                 rhs=wts["wv"][:, k, :],
                                 start=(k == 0), stop=(k == KCH - 1))
            nc.vector.tensor_copy(va[:, :, 0:64],
                                  ps[:].rearrange("p (g d) -> p g d", g=2))
            vas.append(va)

    # --- q projection + attention, one 512-wide q chunk at a time.
    # PE program order per chunk: attention(c), q-proj(c+1), epilogue(c) —
    # the epilogue's DVE chain hides behind the next chunk's projection.
    OTs = [P["persist"].tile([64, Qp], F32, tag=f"oT{h}" + s, name=f"oT{h}" + s)
           for h in (0, 1)]
    NQC = _ceil_div(Qp, 512)

    def emit_qproj(c):
        n = min(512, Qp - c * 512)
        xt = P["x"].tile([128, KCH, n], XDT, tag="xtq", name="xtq", bufs=2)
        nc.sync.dma_start(xt[:], io["xq"][:, :, c * 512:c * 512 + n])
        ps = P["pp"].tile([128, n], F32, tag="pp", name="pp")
        for k in range(KCH):
            nc.tensor.matmul(ps[:], lhsT=wts["wq"][:, k, :], rhs=xt[:, k, :],
                             start=(k == 0), stop=(k == KCH - 1))
        qc = P["persist"].tile([128, n], F32, tag="qT" + s, name="qT" + s,
                               bufs=3)
        # copy on ACT, not DVE: the DVE is busy with the previous chunk's
        # epilogue at this point, and the next chunk's S-matmuls wait on qc
        nc.scalar.copy(qc[:], ps[:])
        return qc

    qcs = {0: emit_qproj(0)}
    for c in range(NQC):
        n = min(512, Qp - c * 512)
        qc = qcs.pop(c)

        otd = [P["ot"].tile([65, n], F32, tag="otd", name="otd") for _ in (0, 1)]

        def emit_s(kt):
            es = []
            for h in (0, 1):
                sps = P["sp"].tile([128, n], F32, tag="sps", name="sps")
                nc.tensor.matmul(
                    sps[:],
                    lhsT=kcs[kt // 4][h * 64:(h + 1) * 64,
                                      (kt % 4) * 128:(kt % 4) * 128 + 128],
                    rhs=qc[h * 64:(h + 1) * 64, :],
                    start=True, stop=True)
                e = P["e"].tile([128, n], F32, tag="e", name="e")
                nc.scalar.activation(e[:], sps[:], EXP,
                                     bias=kb[:, kt:kt + 1], scale=scale)
                es.append(e)
            return es

        # skew-1 software pipeline: S/exp of tile kt+1 issue before the
        # PV matmuls of tile kt, so the PE never waits on the ACT exp
        es_prev = emit_s(0)
        for kt in range(NK):
            es_cur = es_prev
            if kt + 1 < NK:
                es_prev = emit_s(kt + 1)
            for h in (0, 1):
                nc.tensor.matmul(otd[h][:], lhsT=vas[kt][:, h, :],
                                 rhs=es_cur[h][:],
                                 start=(kt == 0), stop=(kt == NK - 1),
                                 skip_group_check=True)
        if c + 1 < NQC:
            qcs[c + 1] = emit_qproj(c + 1)
        for h in (0, 1):
            rrow = P["rows"].tile([65, n], F32, tag="rrow", name="rrow")
            nc.vector.reciprocal(rrow[64:65, :], otd[h][64:65, :])
            nc.vector.tensor_mul(rrow[64:65, :], rrow[64:65, :],
                                 qm[64:65, c * 512:c * 512 + n])
            rps = P["pp"].tile([64, n], F32, tag="pp", name="rps")
            nc.tensor.matmul(rps[:], lhsT=P["ones64"][64:65, 0:64],
                             rhs=rrow[64:65, :], start=True, stop=True)
            rsb = P["rows"].tile([64, n], F32, tag="rsb", name="rsb")
            nc.vector.tensor_copy(rsb[:], rps[:])
            nc.vector.tensor_mul(OTs[h][:, c * 512:c * 512 + n],
                                 otd[h][0:64, :], rsb[:])
    for h in (0, 1):
        nc.sync.dma_start(io["out"][h], OTs[h][:])


def _build_program(phases):
    nc = bacc.Bacc("TRN2", target_bir_lowering=False, debug=False,
                   num_devices=N_CORES)
    for ph in phases:
        s = str(ph["b"])
        Qp, Kp, NK = ph["Qp"], ph["Kp"], ph["NK"]
        io = {
            "xq": nc.dram_tensor("xq" + s, [128, KCH, Qp], XDT, kind="ExternalInput"),
            "xk": nc.dram_tensor("xk" + s, [128, KCH, Kp], XDT, kind="ExternalInput"),
            "xv": nc.dram_tensor("xv" + s, [128, KCH, Kp], XDT, kind="ExternalInput"),
            "kb": nc.dram_tensor("kb" + s, [128, NK], F32, kind="ExternalInput"),
            "qm": nc.dram_tensor("qm" + s, [1, Qp], F32, kind="ExternalInput"),
            "out": nc.dram_tensor("out" + s, [2, 64, Qp], F32, kind="ExternalOutput"),
        }
        ph["io"] = io

    with tile.TileContext(nc) as tc, ExitStack() as ctx:
        P = {
            "w": ctx.enter_context(tc.tile_pool(name="w", bufs=1)),
            "x": ctx.enter_context(tc.tile_pool(name="x", bufs=4)),
            "e": ctx.enter_context(tc.tile_pool(name="e", bufs=8)),
            "rows": ctx.enter_context(tc.tile_pool(name="rows", bufs=2)),
            "persist": ctx.enter_context(tc.tile_pool(name="persist", bufs=1)),
            "pp": ctx.enter_context(tc.tile_pool(name="pp", bufs=2, space="PSUM")),
            "sp": ctx.enter_context(tc.tile_pool(name="sp", bufs=4, space="PSUM")),
            "ot": ctx.enter_context(tc.tile_pool(name="ot", bufs=2, space="PSUM")),
                    }
        ones64 = P["w"].tile([65, 64], F32, tag="ones64", name="ones64")
        nc.vector.memset(ones64[64:65, :], 1.0)
        P["ones64"] = ones64
        warm = P["w"].tile([1, 1], F32, tag="actwarm", name="actwarm")
        nc.vector.memset(warm[:], 0.0)
        nc.scalar.activation(warm[:], warm[:], EXP)
        wts = {}
        for nm in ("wq", "wk", "wv"):
            wd = nc.dram_tensor(nm, [128, KCH, 128], XDT, kind="ExternalInput")
            t = P["w"].tile([128, KCH, 128], XDT, tag=nm, name=nm)
            nc.sync.dma_start(t[:], wd[:])
            wts[nm] = t
        P["wts"] = wts
        for ph in phases:
            _emit_phase(nc, tc, P, ph)
    nc.compile()
    return nc


def _prep_xT(X, P):
    """[T, D] -> [128, KCH, P] with x[p, k, t] = X[t, k*128 + p]."""
    Xp = np.ascontiguousarray(X[:P].T)                 # [D, P]
    return np.ascontiguousarray(
        Xp.reshape(KCH, 128, P).transpose(1, 0, 2)).astype(XNP)  # [128, KCH, P]


def _prep_w(W, c):
    """[D, H*DH] -> per-core [128, KCH, 128] slice of heads (2c, 2c+1)."""
    Ws = W[:, c * 128:(c + 1) * 128]                   # [D, 128]
    return np.ascontiguousarray(
        Ws.reshape(KCH, 128, 128).transpose(1, 0, 2)).astype(XNP)


def kernel(Q_seq, K_seq, V_seq, Q_len, V_len, WQ, WK, WV):
    global LAST_EXEC_NS
    Q_seq = np.asarray(Q_seq, dtype=np.float32)
    K_seq = np.asarray(K_seq, dtype=np.float32)
    V_seq = np.asarray(V_seq, dtype=np.float32)
    WQ = np.asarray(WQ, dtype=np.float32)
    WK = np.asarray(WK, dtype=np.float32)
    WV = np.asarray(WV, dtype=np.float32)
    qlen = [int(np.asarray(Q_len)[b, 0]) for b in range(B)]
    vlen = [int(np.asarray(V_len)[b, 0]) for b in range(B)]

    phases = []
    for b in range(B):
        Qp = _ceil_div(qlen[b], 32) * 32   # q only needs 32-elem alignment
        if Qp == 0:
            continue  # whole batch output is zero
        if vlen[b] > 0:
            NK, scale = _ceil_div(vlen[b], 128), SCALE
        else:
            # all keys masked -> reference softmax degenerates to uniform
            # over all T keys; exp(0*S + 0) = 1 reproduces it exactly.
            NK, scale = T // 128, 0.0
        phases.append(dict(b=b, NQ=_ceil_div(Qp, 128), NK=NK, Qp=Qp,
                           Kp=NK * 128, scale=scale, first=not phases))

    out = np.zeros((B, T, H * DH), dtype=np.float32)
    if not phases:
        return out

    nc = _build_program(phases)

    # per-phase data shared by all cores
    shared = {}
    for ph in phases:
        b, s, Qp, Kp, NK = ph["b"], str(ph["b"]), ph["Qp"], ph["Kp"], ph["NK"]
        kbias = np.where(np.arange(Kp) < vlen[b], 0.0,
                         -NEG_BIG if vlen[b] > 0 else 0.0)
        kbias = np.ascontiguousarray(
            kbias.astype(np.float32).reshape(NK, 128).T)        # [128, NK]
        qmask = (np.arange(Qp) < qlen[b]).astype(np.float32)[None, :]
        shared[s] = {
            "xq" + s: _prep_xT(Q_seq[b], Qp),
            "xk" + s: _prep_xT(K_seq[b], Kp),
            "xv" + s: _prep_xT(V_seq[b], Kp),
            "kb" + s: kbias,
            "qm" + s: np.ascontiguousarray(qmask),
        }

    in_maps = []
    for c in range(N_CORES):
        m = {}
        for ph in phases:
            m.update(shared[str(ph["b"])])
        m["wq"] = _prep_w(WQ, c)
        m["wk"] = _prep_w(WK, c)
        m["wv"] = _prep_w(WV, c)
        in_maps.append(m)

    trace = bool(os.environ.get("BASS_TRACE"))
    if trace:
        _ensure_ntff_hook()
    res = run_bass_kernel_spmd(nc, in_maps, list(range(N_CORES)), trace=trace)
    LAST_EXEC_NS = res.exec_time_ns

    for c in range(N_CORES):
        r = res.results[c]
        for ph in phases:
            b, s, Qp = ph["b"], str(ph["b"]), ph["Qp"]
            o = r["out" + s]  # [2, 64, Qp]
            for h in (0, 1):
                head = 2 * c + h
                out[b, :Qp, head * DH:(head + 1) * DH] = o[h].T
    return out



# revision 9
# speedup vs baseline: 2.1328x; 2.1328x over previous
"""Trainium2 Bass kernel: masked multi-head attention, sharded across 8 NeuronCores.

Problem shapes (hardcoded): B=2, T=2048, D=1024, H=16 heads, dh=64.

Sharding: one SPMD program with two phases (one per batch element). In each
phase every core handles 2 of the 16 heads (core c -> heads 2c, 2c+1), so the
16 heads of each batch are spread over all 8 cores. This load-balances the
data-dependent work (Q_len/V_len trim the q/k tile counts per batch).

v2 changes vs the fp32 baseline:
  - bf16 inputs/weights/intermediates: matmuls run at 1 cycle/row instead of
    fp32's 4 (fp32 lowers to 2 half-speed passes on TRN2), DMA bytes halve.
  - The two heads' S^T matmuls (K=64 each) are row-tiled to disjoint PE
    quadrants (tile_position (0,0)/(64,0)) so they execute concurrently.
  - exp() for both heads merged into one ACT instruction over a 2-bank PSUM
    tile [128, 2, n] (ACT is the #2 engine; fewer/larger instrs).
  - Epilogue: numerator copied once (DVE), softmax denominator row pulled out
    of PSUM by a tiny DMA, reciprocal_approx_fast on DVE (the old
    single-lane RECIPROCAL was 2.2us/chunk), broadcast over partitions with a
    K=1 f32r matmul, one fused multiply per head.
  - Query-length masking moved to the host gather (rows >= Q_len are simply
    not copied out; the output buffer is pre-zeroed) - no qmask work on HW.
  - The second batch's projections are emitted as filler units inside the
    first batch's ACT-paced attention ladder to keep the PE busy.
"""

import math
import os
from collections import deque
from contextlib import ExitStack

import numpy as np
import ml_dtypes

import concourse.bacc as bacc
import concourse.mybir as mybir
import concourse.tile as tile
from concourse.bass_utils import run_bass_kernel_spmd

F32 = mybir.dt.float32
F32R = mybir.dt.float32r
BF16 = mybir.dt.bfloat16
EXP = mybir.ActivationFunctionType.Exp
BNP = ml_dtypes.bfloat16

B, T, D, H, DH = 2, 2048, 1024, 16, 64
N_CORES = 8
KCH = D // 128          # 8 contraction chunks of the model dim
NEG_BIG = 1.0e12
SCALE = 1.0 / math.sqrt(DH)

LAST_EXEC_NS = None     # filled when BASS_TRACE=1


def _ensure_ntff_hook():
    """run_bass_kernel_spmd(trace=True) imports antenv.axon_hooks, which some
    containers lack; synthesize it (backed by libaxon_pjrt's NRT profiling)
    so tracing degrades gracefully instead of crashing."""
    import sys
    import types
    try:
        import antenv.axon_hooks  # noqa: F401
        return
    except ImportError:
        pass
    try:
        import antenv
        from trn_agent_boot.trn_boot import _ntff_profile_via_ctypes
        hook = _ntff_profile_via_ctypes("/opt/axon/libaxon_pjrt.so")
    except Exception:
        antenv = None
        hook = None
    try:
        m = types.ModuleType("antenv.axon_hooks")
        m._hook = hook
        m.set_axon_ntff_profile_hook = lambda h: setattr(m, "_hook", h)
        m.get_axon_ntff_profile_hook = lambda: m._hook
        sys.modules["antenv.axon_hooks"] = m
        if antenv is not None:
            antenv.axon_hooks = m
    except Exception:
        pass


def _ceil_div(a, b):
    return -(-a // b)


def _chunks(total, w=512):
    out = []
    c = 0
    while c < total:
        out.append((c, min(w, total - c)))
        c += w
    return out


class _Emitter:
    def __init__(self, nc, P, wts):
        self.nc = nc
        self.P = P
        self.wts = wts

    # ---------- projection units (each returns nothing, emits instrs) ------

    def kproj_chunk(self, ph, ci):
        """Project keys chunk ci: kc[:, c0:c0+n] = (WK.T @ K_seq.T) slice."""
        nc, P = self.nc, self.P
        c0, n = ph["kch"][ci]
        xt = ph["xk_tiles"][ci]
        ps = P["pp"].tile([128, 512], F32, tag="pp", name="kps")
        for k in range(KCH):
            nc.tensor.matmul(ps[:, :n], lhsT=self.wts["wk"][:, k, :],
                             rhs=xt[:, k, :n],
                             start=(k == 0), stop=(k == KCH - 1),
                             skip_group_check=True)
        nc.vector.tensor_copy(ph["kc"][:, c0:c0 + n], ps[:, :n])

    def vproj_tile(self, ph, m):
        """Project value tokens [m*128,(m+1)*128) into va[:, m, :, 0:64]."""
        nc, P = self.nc, self.P
        ci, r = divmod(m * 128, 512)
        c0, cn = ph["vch"][ci]
        xt = ph["xv_tiles"][ci]
        ps = P["pp"].tile([128, 512], F32, tag="pp", name="vps")
        for k in range(KCH):
            nc.tensor.matmul(ps[:, 0:128], lhsT=xt[:, k, r:r + 128],
                             rhs=self.wts["wv"][:, k, :],
                             start=(k == 0), stop=(k == KCH - 1),
                             skip_group_check=True)
        nc.vector.tensor_copy(
            ph["va"][:, m, :, 0:64],
            ps[:, 0:128].rearrange("p (g d) -> p g d", g=2))

    def qproj_chunk(self, ph, ci):
        """Project queries chunk ci into the qc ring; returns the tile."""
        nc, P = self.nc, self.P
        c0, n = ph["qch"][ci]
        xt = ph["xq_tiles"][ci]
        ps = P["pp"].tile([128, 512], F32, tag="pp", name="qps")
        for k in range(KCH):
            nc.tensor.matmul(ps[:, :n], lhsT=self.wts["wq"][:, k, :],
                             rhs=xt[:, k, :n],
                             start=(k == 0), stop=(k == KCH - 1),
                             skip_group_check=True)
        qc = P["qc"].tile([128, 512], BF16, tag="qc" + str(ph["b"]),
                          name="qc", bufs=3)
        nc.vector.tensor_copy(qc[:, :n], ps[:, :n])
        ph["qcs"][ci] = qc
        return qc

    # ---------- attention ladder ------------------------------------------

    def ladder(self, ph, ci, fillers):
        """S/exp/PV software pipeline for q chunk ci; pops one filler unit
        (a closure emitting an independent projection) per key-tile step."""
        nc, P = self.nc, self.P
        c0, n = ph["qch"][ci]
        NK = ph["NK"]
        qc = ph["qcs"].pop(ci)
        kb = ph["kb_tile"]
        kc, va = ph["kc"], ph["va"]
        scale = ph["scale"]

        otd = P["ot"].tile([65, 2, 512], F32, tag="ot", name="otd")

        def emit_s(kt):
            sps = P["sp"].tile([128, 2, 512], F32, tag="sp", name="sps")
            for h in (0, 1):
                nc.tensor.matmul(
                    sps[:, h, :n],
                    lhsT=kc[h * 64:(h + 1) * 64, kt * 128:(kt + 1) * 128],
                    rhs=qc[h * 64:(h + 1) * 64, :n],
                    start=True, stop=True,
                    tile_position=(h * 64, 0),
                    skip_group_check=True)
            e = P["e"].tile([128, 2, 512], BF16, tag="e", name="e", bufs=3)
            nc.scalar.activation(e[:, :, :n], sps[:, :, :n], EXP,
                                 bias=kb[:, kt:kt + 1], scale=scale)
            return e

        ep = emit_s(0)
        for kt in range(NK):
            ec = ep
            if kt + 1 < NK:
                ep = emit_s(kt + 1)
            if fillers:
                fillers.popleft()()
            for h in (0, 1):
                nc.tensor.matmul(otd[:, h, :n], lhsT=va[:, kt, h, :],
                                 rhs=ec[:, h, :n],
                                 start=(kt == 0), stop=(kt == NK - 1),
                                 skip_group_check=True)
        return otd

    def epilogue(self, ph, ci, otd):
        """Normalize otd -> OTs[:, :, c0:c0+n] (no qmask: host trims)."""
        nc, P = self.nc, self.P
        c0, n = ph["qch"][ci]
        ou = P["ou"].tile([64, 2, 512], BF16, tag="ou", name="ou", bufs=2)
        nc.vector.tensor_copy(ou[:, :, :n], otd[0:64, :, :n])
        drow = P["rows"].tile([65, 2, 512], BF16, tag="drow", name="drow",
                              bufs=2)
        nc.scalar.copy(drow[64:65, :, :n], otd[64:65, :, :n])
        rsb = P["rows"].tile([64, 2, 512], F32, tag="rsb", name="rsb",
                             bufs=2)
        for h in (0, 1):
            # broadcast d over 64 partitions (K=1 bf16 matmul), then
            # reciprocal on the [64, n] block (DVE cost is free-size-based,
            # so this is no dearer than a single-partition reciprocal).
            dps = P["pp"].tile([128, 512], F32, tag="pp", name="dps")
            nc.tensor.matmul(dps[0:64, :n],
                             lhsT=P["onesr"][64:65, 0:64],
                             rhs=drow[64:65, h, :n],
                             start=True, stop=True, skip_group_check=True)
            nc.vector.reciprocal_approx_fast(rsb[:, h, :n], dps[0:64, :n])
            nc.vector.tensor_mul(ph["OTs"][:, h, c0:c0 + n],
                                 ou[:, h, :n], rsb[:, h, :n])


def _filler_units(em, ph, skip_k0=True, skip_v=(), with_qproj=False):
    """(deadline, closure) units for a phase's projections. Deadlines are in
    key-tile steps of the ladder they'll be popped in (None = no deadline)."""
    units = []
    for ci in range(len(ph["kch"])):
        if skip_k0 and ci == 0:
            continue
        d = (ph["kch"][ci][0] // 128) - 2 if skip_k0 else None
        units.append((d, lambda ci=ci: em.kproj_chunk(ph, ci)))
    for m in range(ph["NK"]):
        if m in skip_v:
            continue
        d = m if skip_k0 else None
        units.append((d, lambda m=m: em.vproj_tile(ph, m)))
    if with_qproj:
        units.append((None, lambda: em.qproj_chunk(ph, 0)))
    return units


def _edf_pack(units, nsteps):
    """Earliest-deadline-first: assign units to ladder steps; any unit that
    would miss its deadline is returned in `pre` (emit before the ladder)."""
    units = sorted(units, key=lambda u: (u[0] is None, u[0] or 0))
    pre, sched, step = [], deque(), 0
    for d, fn in units:
        if step < nsteps and (d is None or step <= d):
            sched.append(fn)
            step += 1
        elif d is not None:
            pre.append(fn)
        else:
            sched.append(fn)  # deadline-free overflow: popped by later chunks
    return pre, sched


def _build_program(phases):
    nc = bacc.Bacc("TRN2", target_bir_lowering=False, debug=False,
                   num_devices=N_CORES)
    for ph in phases:
        s = str(ph["b"])
        Qp, Kp, NK = ph["Qp"], ph["Kp"], ph["NK"]
        ph["io"] = {
            "xq": nc.dram_tensor("xq" + s, [128, KCH, Qp], BF16, kind="ExternalInput"),
            "xk": nc.dram_tensor("xk" + s, [128, KCH, Kp], BF16, kind="ExternalInput"),
            "xv": nc.dram_tensor("xv" + s, [128, KCH, Kp], BF16, kind="ExternalInput"),
            "kb": nc.dram_tensor("kb" + s, [128, NK], F32, kind="ExternalInput"),
            "out": nc.dram_tensor("out" + s, [64, 2, Qp], BF16, kind="ExternalOutput"),
        }
        ph["qch"] = _chunks(Qp)
        ph["kch"] = _chunks(Kp)
        ph["vch"] = ph["kch"]
        ph["qcs"] = {}

    with tile.TileContext(nc) as tc, ExitStack() as ctx:
        P = {
            "w": ctx.enter_context(tc.tile_pool(name="w", bufs=1)),
            "x": ctx.enter_context(tc.tile_pool(name="x", bufs=1)),
            "qc": ctx.enter_context(tc.tile_pool(name="qc", bufs=3)),
            "e": ctx.enter_context(tc.tile_pool(name="e", bufs=3)),
            "ou": ctx.enter_context(tc.tile_pool(name="ou", bufs=2)),
            "rows": ctx.enter_context(tc.tile_pool(name="rows", bufs=2)),
            "persist": ctx.enter_context(tc.tile_pool(name="persist", bufs=1)),
            "pp": ctx.enter_context(tc.tile_pool(name="pp", bufs=2, space="PSUM")),
            "sp": ctx.enter_context(tc.tile_pool(name="sp", bufs=2, space="PSUM")),
            "ot": ctx.enter_context(tc.tile_pool(name="ot", bufs=1, space="PSUM")),
        }
        onesr = P["w"].tile([65, 64], BF16, tag="onesr", name="onesr")
        nc.vector.memset(onesr[64:65, :], 1.0)
        P["onesr"] = onesr
        warm = P["w"].tile([1, 1], F32, tag="actwarm", name="actwarm")
        nc.vector.memset(warm[:], 0.0)
        nc.scalar.activation(warm[:], warm[:], EXP)

        # PE p-state warmup: dummy bf16 matmuls on zeroed tiles keep the PE
        # clocking up while the first input DMAs land.
        zw = P["w"].tile([128, 128], BF16, tag="zw", name="zw")
        nc.gpsimd.memset(zw[:], 0.0)
        zw2 = P["w"].tile([128, 512], BF16, tag="zw2", name="zw2")
        nc.gpsimd.memset(zw2[:], 0.0)
        for _ in range(2):
            wps = P["sp"].tile([128, 2, 512], F32, tag="sp", name="wps")
            for r in range(4):
                nc.tensor.matmul(wps[:, 0, :], lhsT=zw[:], rhs=zw2[:],
                                 start=(r == 0), stop=(r == 3),
                                 skip_group_check=True)

        # -------- weights --------
        wts = {}
        for nm in ("wk", "wq", "wv"):
            wd = nc.dram_tensor(nm, [128, KCH, 128], BF16, kind="ExternalInput")
            t = P["w"].tile([128, KCH, 128], BF16, tag=nm, name=nm)
            nc.sync.dma_start(t[:], wd[:])
            wts[nm] = t

        # -------- input staging (all up-front; queue order = priority) ----
        A = phases[0]
        Bp = phases[1] if len(phases) > 1 else None
        for ph in phases:
            s = str(ph["b"])
            kb = P["w"].tile([128, ph["NK"]], F32, tag="kb" + s, name="kb")
            nc.sync.dma_start(kb[:], ph["io"]["kb"][:])
            ph["kb_tile"] = kb

        def stage(ph, key, chl, slice_first=False):
            s = str(ph["b"])
            tiles = []
            for ci, (c0, n) in enumerate(chl):
                xt = P["x"].tile([128, KCH, 512], BF16, tag=f"{key}{s}",
                                 name=f"{key}{s}", bufs=len(chl))
                if slice_first and ci == 0:
                    for k in range(KCH):
                        nc.sync.dma_start(xt[:, k, :n],
                                          ph["io"][key][:, k, c0:c0 + n])
                else:
                    nc.sync.dma_start(xt[:, :, :n],
                                      ph["io"][key][:, :, c0:c0 + n])
                tiles.append(xt)
            ph[f"{key}_tiles"] = tiles

        stage(A, "xk", A["kch"], slice_first=True)
        stage(A, "xq", A["qch"])
        stage(A, "xv", A["vch"])
        if Bp is not None:
            stage(Bp, "xk", Bp["kch"])
            stage(Bp, "xv", Bp["vch"])
            stage(Bp, "xq", Bp["qch"])

        # -------- persistent per-phase tiles --------
        for ph in phases:
            s = str(ph["b"])
            ph["kc"] = P["persist"].tile([128, ph["Kp"]], BF16,
                                         tag="kc" + s, name="kc" + s)
            ph["va"] = P["persist"].tile([128, ph["NK"], 2, 65], BF16,
                                         tag="va" + s, name="va" + s)
            nc.gpsimd.memset(ph["va"][:, :, :, 64:65], 1.0)
            ph["OTs"] = P["persist"].tile([64, 2, ph["Qp"]], BF16,
                                          tag="oT" + s, name="oT" + s)

        em = _Emitter(nc, P, wts)

        # -------- phase A flow --------
        em.kproj_chunk(A, 0)
        em.qproj_chunk(A, 0)
        pre, sched = _edf_pack(_filler_units(em, A, skip_k0=True),
                               A["NK"])
        for fn in pre:
            fn()
        if Bp is not None:
            bunits = _filler_units(em, Bp, skip_k0=False, with_qproj=True)
            sched.extend(fn for _, fn in bunits)

        for ci in range(len(A["qch"])):
            otd = em.ladder(A, ci, sched)
            if ci + 1 < len(A["qch"]):
                em.qproj_chunk(A, ci + 1)
            em.epilogue(A, ci, otd)
        nc.sync.dma_start(A["io"]["out"][:], A["OTs"][:])

        # -------- phase B flow --------
        if Bp is not None:
            while sched:
                sched.popleft()()
            if 0 not in Bp["qcs"]:
                em.qproj_chunk(Bp, 0)
            for ci in range(len(Bp["qch"])):
                otd = em.ladder(Bp, ci, sched)
                if ci + 1 < len(Bp["qch"]):
                    em.qproj_chunk(Bp, ci + 1)
                em.epilogue(Bp, ci, otd)
            nc.sync.dma_start(Bp["io"]["out"][:], Bp["OTs"][:])

    nc.compile()
    return nc


def _prep_xT(X, Pq):
    """[T, D] -> [128, KCH, Pq] bf16 with x[p, k, t] = X[t, k*128 + p]."""
    Xp = np.ascontiguousarray(X[:Pq].T)                 # [D, Pq]
    return np.ascontiguousarray(
        Xp.reshape(KCH, 128, Pq).transpose(1, 0, 2)).astype(BNP)


def _prep_w(W, c):
    """[D, H*DH] -> per-core [128, KCH, 128] bf16 slice of heads (2c, 2c+1)."""
    Ws = W[:, c * 128:(c + 1) * 128]                    # [D, 128]
    return np.ascontiguousarray(
        Ws.reshape(KCH, 128, 128).transpose(1, 0, 2)).astype(BNP)


def kernel(Q_seq, K_seq, V_seq, Q_len, V_len, WQ, WK, WV):
    global LAST_EXEC_NS
    Q_seq = np.asarray(Q_seq, dtype=np.float32)
    K_seq = np.asarray(K_seq, dtype=np.float32)
    V_seq = np.asarray(V_seq, dtype=np.float32)
    WQ = np.asarray(WQ, dtype=np.float32)
    WK = np.asarray(WK, dtype=np.float32)
    WV = np.asarray(WV, dtype=np.float32)
    qlen = [int(np.asarray(Q_len)[b, 0]) for b in range(B)]
    vlen = [int(np.asarray(V_len)[b, 0]) for b in range(B)]

    phases = []
    for b in range(B):
        Qp = _ceil_div(qlen[b], 32) * 32   # q only needs 32-elem alignment
        if Qp == 0:
            continue  # whole batch output is zero
        if vlen[b] > 0:
            NK, scale = _ceil_div(vlen[b], 128), SCALE
        else:
            # all keys masked -> reference softmax degenerates to uniform
            # over all T keys; exp(0*S + 0) = 1 reproduces it exactly.
            NK, scale = T // 128, 0.0
        phases.append(dict(b=b, NK=NK, Qp=Qp, Kp=NK * 128, scale=scale))
    phases.sort(key=lambda ph: -ph["Qp"])  # big phase first (filler donor)

    out = np.zeros((B, T, H * DH), dtype=np.float32)
    if not phases:
        return out

    nc = _build_program(phases)

    # per-phase data shared by all cores
    shared = {}
    for ph in phases:
        b, s, Qp, Kp, NK = ph["b"], str(ph["b"]), ph["Qp"], ph["Kp"], ph["NK"]
        kbias = np.where(np.arange(Kp) < vlen[b], 0.0,
                         -NEG_BIG if vlen[b] > 0 else 0.0)
        kbias = np.ascontiguousarray(
            kbias.astype(np.float32).reshape(NK, 128).T)        # [128, NK]
        shared[s] = {
            "xq" + s: _prep_xT(Q_seq[b], Qp),
            "xk" + s: _prep_xT(K_seq[b], Kp),
            "xv" + s: _prep_xT(V_seq[b], Kp),
            "kb" + s: kbias,
        }

    in_maps = []
    for c in range(N_CORES):
        m = {}
        for ph in phases:
            m.update(shared[str(ph["b"])])
        m["wq"] = _prep_w(WQ, c)
        m["wk"] = _prep_w(WK, c)
        m["wv"] = _prep_w(WV, c)
        in_maps.append(m)

    trace = bool(os.environ.get("BASS_TRACE"))
    if trace:
        _ensure_ntff_hook()
    res = run_bass_kernel_spmd(nc, in_maps, list(range(N_CORES)), trace=trace)
    LAST_EXEC_NS = res.exec_time_ns

    for c in range(N_CORES):
        r = res.results[c]
        for ph in phases:
            b, s, ql = ph["b"], str(ph["b"]), qlen[ph["b"]]
            o = np.asarray(r["out" + s]).astype(np.float32)  # [64, 2, Qp]
            for h in (0, 1):
                head = 2 * c + h
                out[b, :ql, head * DH:(head + 1) * DH] = o[:, h, :ql].T
    return out


# revision 14
# speedup vs baseline: 2.2671x; 1.0630x over previous
"""Trainium2 Bass kernel: masked multi-head attention, sharded across 8 NeuronCores.

Problem shapes (hardcoded): B=2, T=2048, D=1024, H=16 heads, dh=64.

Sharding: one SPMD program with two phases (one per batch element). In each
phase every core handles 2 of the 16 heads (core c -> heads 2c, 2c+1), so the
16 heads of each batch are spread over all 8 cores. This load-balances the
data-dependent work (Q_len/V_len trim the q/k tile counts per batch).

v2 changes vs the fp32 baseline:
  - bf16 inputs/weights/intermediates: matmuls run at 1 cycle/row instead of
    fp32's 4 (fp32 lowers to 2 half-speed passes on TRN2), DMA bytes halve.
  - The two heads' S^T matmuls (K=64 each) are row-tiled to disjoint PE
    quadrants (tile_position (0,0)/(64,0)) so they execute concurrently.
  - exp() for both heads merged into one ACT instruction over a 2-bank PSUM
    tile [128, 2, n] (ACT is the #2 engine; fewer/larger instrs).
  - Epilogue: numerator copied once (DVE), softmax denominator row pulled out
    of PSUM by a tiny DMA, reciprocal_approx_fast on DVE (the old
    single-lane RECIPROCAL was 2.2us/chunk), broadcast over partitions with a
    K=1 f32r matmul, one fused multiply per head.
  - Query-length masking moved to the host gather (rows >= Q_len are simply
    not copied out; the output buffer is pre-zeroed) - no qmask work on HW.
  - The second batch's projections are emitted as filler units inside the
    first batch's ACT-paced attention ladder to keep the PE busy.
"""

import math
import os
from collections import deque
from contextlib import ExitStack

import numpy as np
import ml_dtypes

import concourse.bacc as bacc
import concourse.mybir as mybir
import concourse.tile as tile
from concourse.bass_utils import run_bass_kernel_spmd

F32 = mybir.dt.float32
F32R = mybir.dt.float32r
BF16 = mybir.dt.bfloat16
EXP = mybir.ActivationFunctionType.Exp
BNP = ml_dtypes.bfloat16

B, T, D, H, DH = 2, 2048, 1024, 16, 64
N_CORES = 8
KCH = D // 128          # 8 contraction chunks of the model dim
NEG_BIG = 1.0e12
SCALE = 1.0 / math.sqrt(DH)

LAST_EXEC_NS = None     # filled when BASS_TRACE=1


def _ensure_ntff_hook():
    """run_bass_kernel_spmd(trace=True) imports antenv.axon_hooks, which some
    containers lack; synthesize it (backed by libaxon_pjrt's NRT profiling)
    so tracing degrades gracefully instead of crashing."""
    import sys
    import types
    try:
        import antenv.axon_hooks  # noqa: F401
        return
    except ImportError:
        pass
    try:
        import antenv
        from trn_agent_boot.trn_boot import _ntff_profile_via_ctypes
        hook = _ntff_profile_via_ctypes("/opt/axon/libaxon_pjrt.so")
    except Exception:
        antenv = None
        hook = None
    try:
        m = types.ModuleType("antenv.axon_hooks")
        m._hook = hook
        m.set_axon_ntff_profile_hook = lambda h: setattr(m, "_hook", h)
        m.get_axon_ntff_profile_hook = lambda: m._hook
        sys.modules["antenv.axon_hooks"] = m
        if antenv is not None:
            antenv.axon_hooks = m
    except Exception:
        pass


def _ceil_div(a, b):
    return -(-a // b)


def _chunks(total, w=512):
    out = []
    c = 0
    while c < total:
        out.append((c, min(w, total - c)))
        c += w
    return out


class _Emitter:
    def __init__(self, nc, P, wts):
        self.nc = nc
        self.P = P
        self.wts = wts

    # ---------- projection units (each returns nothing, emits instrs) ------

    def kproj_chunk(self, ph, ci):
        """Project keys chunk ci: kc[:, c0:c0+n] = (WK.T @ K_seq.T) slice."""
        nc, P = self.nc, self.P
        c0, n = ph["kch"][ci]
        xt = ph["xk_tiles"][ci]
        ps = P["pp"].tile([128, 512], F32, tag="pp", name="kps")
        for k in range(KCH):
            nc.tensor.matmul(ps[:, :n], lhsT=self.wts["wk"][:, k, :],
                             rhs=xt[:, k, :n],
                             start=(k == 0), stop=(k == KCH - 1),
                             skip_group_check=True)
        nc.vector.tensor_copy(ph["kc"][:, c0:c0 + n], ps[:, :n])

    def vproj_tile(self, ph, m):
        """Project value tokens [m*128,(m+1)*128) into va[:, m, :, 0:64]."""
        nc, P = self.nc, self.P
        ci, r = divmod(m * 128, 512)
        c0, cn = ph["vch"][ci]
        xt = ph["xv_tiles"][ci]
        ps = P["pp"].tile([128, 512], F32, tag="pp", name="vps")
        for k in range(KCH):
            nc.tensor.matmul(ps[:, 0:128], lhsT=xt[:, k, r:r + 128],
                             rhs=self.wts["wv"][:, k, :],
                             start=(k == 0), stop=(k == KCH - 1),
                             skip_group_check=True)
        nc.vector.tensor_copy(
            ph["va"][:, m, :, 0:64],
            ps[:, 0:128].rearrange("p (g d) -> p g d", g=2))

    def qproj_chunk(self, ph, ci):
        """Project queries chunk ci into the qc ring; returns the tile."""
        nc, P = self.nc, self.P
        c0, n = ph["qch"][ci]
        xt = ph["xq_tiles"][ci]
        ps = P["pp"].tile([128, 512], F32, tag="pp", name="qps")
        for k in range(KCH):
            nc.tensor.matmul(ps[:, :n], lhsT=self.wts["wq"][:, k, :],
                             rhs=xt[:, k, :n],
                             start=(k == 0), stop=(k == KCH - 1),
                             skip_group_check=True)
        qc = P["qc"].tile([128, 512], BF16, tag="qc" + str(ph["b"]),
                          name="qc", bufs=3)
        nc.vector.tensor_copy(qc[:, :n], ps[:, :n])
        ph["qcs"][ci] = qc
        return qc

    # ---------- attention ladder ------------------------------------------

    def ladder(self, ph, ci, fillers):
        """S/exp/PV software pipeline for q chunk ci; pops one filler unit
        (a closure emitting an independent projection) per key-tile step."""
        nc, P = self.nc, self.P
        c0, n = ph["qch"][ci]
        NK = ph["NK"]
        qc = ph["qcs"].pop(ci)
        kb = ph["kb_tile"]
        kc, va = ph["kc"], ph["va"]
        scale = ph["scale"]

        otd = P["ot"].tile([65, 2, 512], F32, tag="ot", name="otd")

        def emit_s(kt):
            sps = P["sp"].tile([128, 2, 512], F32, tag="sp", name="sps")
            for h in (0, 1):
                nc.tensor.matmul(
                    sps[:, h, :n],
                    lhsT=kc[h * 64:(h + 1) * 64, kt * 128:(kt + 1) * 128],
                    rhs=qc[h * 64:(h + 1) * 64, :n],
                    start=True, stop=True,
                    tile_position=(h * 64, 0),
                    skip_group_check=True)
            e = P["e"].tile([128, 2, 512], BF16, tag="e", name="e", bufs=3)
            nc.scalar.activation(e[:, :, :n], sps[:, :, :n], EXP,
                                 bias=kb[:, kt:kt + 1], scale=scale)
            return e

        ep = emit_s(0)
        for kt in range(NK):
            ec = ep
            if kt + 1 < NK:
                ep = emit_s(kt + 1)
            if fillers:
                fillers.popleft()()
            for h in (0, 1):
                nc.tensor.matmul(otd[:, h, :n], lhsT=va[:, kt, h, :],
                                 rhs=ec[:, h, :n],
                                 start=(kt == 0), stop=(kt == NK - 1),
                                 skip_group_check=True)
        return otd

    def epilogue(self, ph, ci, otd):
        """Normalize otd -> OTs[:, :, c0:c0+n] (no qmask: host trims)."""
        nc, P = self.nc, self.P
        c0, n = ph["qch"][ci]
        ou = P["ou"].tile([64, 2, 512], BF16, tag="ou", name="ou", bufs=2)
        nc.vector.tensor_copy(ou[:, :, :n], otd[0:64, :, :n])
        drow = P["rows"].tile([65, 2, 512], BF16, tag="drow", name="drow",
                              bufs=2)
        nc.vector.tensor_copy(drow[64:65, :, :n], otd[64:65, :, :n])
        rsb = P["rows"].tile([64, 2, 512], F32, tag="rsb", name="rsb",
                             bufs=2)
        for h in (0, 1):
            # broadcast d over 64 partitions (K=1 bf16 matmul), then
            # reciprocal on the [64, n] block (DVE cost is free-size-based,
            # so this is no dearer than a single-partition reciprocal).
            dps = P["pp"].tile([128, 512], F32, tag="pp", name="dps")
            nc.tensor.matmul(dps[0:64, :n],
                             lhsT=P["onesr"][64:65, 0:64],
                             rhs=drow[64:65, h, :n],
                             start=True, stop=True, skip_group_check=True)
            nc.vector.reciprocal_approx_fast(rsb[:, h, :n], dps[0:64, :n])
            nc.vector.tensor_mul(ph["OTs"][:, h, c0:c0 + n],
                                 ou[:, h, :n], rsb[:, h, :n])


def _filler_units(em, ph, skip_k0=True, skip_v=(), with_qproj=False):
    """(deadline, closure) units for a phase's projections. Deadlines are in
    key-tile steps of the ladder they'll be popped in (None = no deadline)."""
    units = []
    for ci in range(len(ph["kch"])):
        if skip_k0 and ci == 0:
            continue
        d = (ph["kch"][ci][0] // 128) - 2 if skip_k0 else None
        units.append((d, lambda ci=ci: em.kproj_chunk(ph, ci)))
    for m in range(ph["NK"]):
        if m in skip_v:
            continue
        d = m if skip_k0 else None
        units.append((d, lambda m=m: em.vproj_tile(ph, m)))
    if with_qproj:
        units.append((None, lambda: em.qproj_chunk(ph, 0)))
    return units


def _edf_pack(units, nsteps):
    """Earliest-deadline-first: assign units to ladder steps; any unit that
    would miss its deadline is returned in `pre` (emit before the ladder)."""
    units = sorted(units, key=lambda u: (u[0] is None, u[0] or 0))
    pre, sched, step = [], deque(), 0
    for d, fn in units:
        if step < nsteps and (d is None or step <= d):
            sched.append(fn)
            step += 1
        elif d is not None:
            pre.append(fn)
        else:
            sched.append(fn)  # deadline-free overflow: popped by later chunks
    return pre, sched


def _build_program(phases):
    nc = bacc.Bacc("TRN2", target_bir_lowering=False, debug=False,
                   num_devices=N_CORES)
    for ph in phases:
        s = str(ph["b"])
        Qp, Kp, NK = ph["Qp"], ph["Kp"], ph["NK"]
        ph["qch"] = _chunks(Qp)
        ph["kch"] = _chunks(Kp)
        ph["vch"] = ph["kch"]
        ph["qcs"] = {}
        io = {
            "kb": nc.dram_tensor("kb" + s, [128, NK], F32, kind="ExternalInput"),
            "out": nc.dram_tensor("out" + s, [64, 2, Qp], BF16, kind="ExternalOutput"),
        }
        # per-chunk input tensors: per-partition-contiguous so each DMA
        # lowers to 128 large descriptors instead of 1KB-strided fragments
        for key, chl in (("xq", ph["qch"]), ("xk", ph["kch"]), ("xv", ph["vch"])):
            for ci, (c0, n) in enumerate(chl):
                io[f"{key}c{ci}"] = nc.dram_tensor(
                    f"{key}{s}c{ci}", [128, KCH, n], BF16, kind="ExternalInput")
        ph["io"] = io

    with tile.TileContext(nc) as tc, ExitStack() as ctx:
        P = {
            "w": ctx.enter_context(tc.tile_pool(name="w", bufs=1)),
            "x": ctx.enter_context(tc.tile_pool(name="x", bufs=1)),
            "qc": ctx.enter_context(tc.tile_pool(name="qc", bufs=3)),
            "e": ctx.enter_context(tc.tile_pool(name="e", bufs=3)),
            "ou": ctx.enter_context(tc.tile_pool(name="ou", bufs=2)),
            "rows": ctx.enter_context(tc.tile_pool(name="rows", bufs=2)),
            "persist": ctx.enter_context(tc.tile_pool(name="persist", bufs=1)),
            "pp": ctx.enter_context(tc.tile_pool(name="pp", bufs=2, space="PSUM")),
            "sp": ctx.enter_context(tc.tile_pool(name="sp", bufs=2, space="PSUM")),
            "ot": ctx.enter_context(tc.tile_pool(name="ot", bufs=1, space="PSUM")),
        }
        onesr = P["w"].tile([65, 64], BF16, tag="onesr", name="onesr")
        nc.vector.memset(onesr[64:65, :], 1.0)
        P["onesr"] = onesr
        warm = P["w"].tile([1, 1], F32, tag="actwarm", name="actwarm")
        nc.vector.memset(warm[:], 0.0)
        nc.scalar.activation(warm[:], warm[:], EXP)

        # PE p-state warmup: dummy bf16 matmuls on zeroed tiles keep the PE
        # clocking up while the first input DMAs land.
        zw = P["w"].tile([128, 128], BF16, tag="zw", name="zw")
        nc.gpsimd.memset(zw[:], 0.0)
        zw2 = P["w"].tile([128, 512], BF16, tag="zw2", name="zw2")
        nc.gpsimd.memset(zw2[:], 0.0)
        for _ in range(2):
            wps = P["sp"].tile([128, 2, 512], F32, tag="sp", name="wps")
            for r in range(4):
                nc.tensor.matmul(wps[:, 0, :], lhsT=zw[:], rhs=zw2[:],
                                 start=(r == 0), stop=(r == 3),
                                 skip_group_check=True)

        # -------- weights --------
        wts = {}
        for nm in ("wk", "wq", "wv"):
            wd = nc.dram_tensor(nm, [128, KCH, 128], BF16, kind="ExternalInput")
            t = P["w"].tile([128, KCH, 128], BF16, tag=nm, name=nm)
            nc.sync.dma_start(t[:], wd[:])
            wts[nm] = t

        # -------- input staging (issue order == consumption order) --------
        A = phases[0]
        Bp = phases[1] if len(phases) > 1 else None
        for ph in phases:
            s = str(ph["b"])
            kb = P["w"].tile([128, ph["NK"]], F32, tag="kb" + s, name="kb")
            nc.sync.dma_start(kb[:], ph["io"]["kb"][:])
            ph["kb_tile"] = kb
            for key, chl in (("xq", ph["qch"]), ("xk", ph["kch"]),
                             ("xv", ph["vch"])):
                ph[f"{key}_tiles"] = [None] * len(chl)

        def stage1(ph, key, ci, sliced=False):
            s = str(ph["b"])
            n = dict(xq=ph["qch"], xk=ph["kch"], xv=ph["vch"])[key][ci][1]
            xt = P["x"].tile([128, KCH, n], BF16, tag=f"{key}{s}c{ci}",
                             name=f"{key}{s}c{ci}", bufs=1)
            src = ph["io"][f"{key}c{ci}"]
            if sliced:
                for k in range(KCH):
                    nc.sync.dma_start(xt[:, k, :], src[:, k, :])
            else:
                nc.sync.dma_start(xt[:], src[:])
            ph[f"{key}_tiles"][ci] = xt

        stage1(A, "xk", 0, sliced=True)
        stage1(A, "xq", 0)
        stage1(A, "xv", 0)
        for ci in range(1, len(A["kch"])):
            stage1(A, "xk", ci)
            stage1(A, "xv", ci)
        for ci in range(1, len(A["qch"])):
            stage1(A, "xq", ci)
        if Bp is not None:
            for ci in range(len(Bp["kch"])):
                stage1(Bp, "xk", ci)
                stage1(Bp, "xv", ci)
            for ci in range(len(Bp["qch"])):
                stage1(Bp, "xq", ci)

        # -------- persistent per-phase tiles --------
        for ph in phases:
            s = str(ph["b"])
            ph["kc"] = P["persist"].tile([128, ph["Kp"]], BF16,
                                         tag="kc" + s, name="kc" + s)
            ph["va"] = P["persist"].tile([128, ph["NK"], 2, 65], BF16,
                                         tag="va" + s, name="va" + s)
            nc.gpsimd.memset(ph["va"][:, :, :, 64:65], 1.0)
            ph["OTs"] = P["persist"].tile([64, 2, ph["Qp"]], BF16,
                                          tag="oT" + s, name="oT" + s)

        em = _Emitter(nc, P, wts)

        # -------- phase A flow --------
        em.kproj_chunk(A, 0)
        em.qproj_chunk(A, 0)
        nqA = len(A["qch"])
        u0 = _filler_units(em, A, skip_k0=True)
        if nqA > 1:
            u0.append((A["NK"] - 1, lambda: em.qproj_chunk(A, 1)))
        pre, q0 = _edf_pack(u0, A["NK"])
        for fn in pre:
            fn()
        # later chunks: qproj(ci+1) first, then phase-B units round-robin
        queues = [q0] + [deque() for _ in range(max(0, nqA - 1))]
        for ci in range(1, nqA - 1):
            queues[ci].append(lambda ci=ci: em.qproj_chunk(A, ci + 1))
        rest = deque()
        if Bp is not None:
            rest.extend(fn for _, fn in
                        _filler_units(em, Bp, skip_k0=False, with_qproj=True))
        for ci in range(1, nqA):
            cap = A["NK"] - len(queues[ci])
            while rest and cap > 0:
                queues[ci].append(rest.popleft())
                cap -= 1

        for ci in range(nqA):
            otd = em.ladder(A, ci, queues[ci])
            em.epilogue(A, ci, otd)
        nc.sync.dma_start(A["io"]["out"][:], A["OTs"][:])

        # -------- phase B flow --------
        if Bp is not None:
            while rest:
                rest.popleft()()
            if 0 not in Bp["qcs"]:
                em.qproj_chunk(Bp, 0)
            for ci in range(len(Bp["qch"])):
                otd = em.ladder(Bp, ci, deque())
                if ci + 1 < len(Bp["qch"]):
                    em.qproj_chunk(Bp, ci + 1)
                em.epilogue(Bp, ci, otd)
            nc.sync.dma_start(Bp["io"]["out"][:], Bp["OTs"][:])

    nc.compile()
    return nc


def _prep_xT(X, Pq):
    """[T, D] -> [128, KCH, Pq] bf16 with x[p, k, t] = X[t, k*128 + p]."""
    Xp = np.ascontiguousarray(X[:Pq].T)                 # [D, Pq]
    return np.ascontiguousarray(
        Xp.reshape(KCH, 128, Pq).transpose(1, 0, 2)).astype(BNP)


def _prep_w(W, c):
    """[D, H*DH] -> per-core [128, KCH, 128] bf16 slice of heads (2c, 2c+1)."""
    Ws = W[:, c * 128:(c + 1) * 128]                    # [D, 128]
    return np.ascontiguousarray(
        Ws.reshape(KCH, 128, 128).transpose(1, 0, 2)).astype(BNP)


def kernel(Q_seq, K_seq, V_seq, Q_len, V_len, WQ, WK, WV):
    global LAST_EXEC_NS
    Q_seq = np.asarray(Q_seq, dtype=np.float32)
    K_seq = np.asarray(K_seq, dtype=np.float32)
    V_seq = np.asarray(V_seq, dtype=np.float32)
    WQ = np.asarray(WQ, dtype=np.float32)
    WK = np.asarray(WK, dtype=np.float32)
    WV = np.asarray(WV, dtype=np.float32)
    qlen = [int(np.asarray(Q_len)[b, 0]) for b in range(B)]
    vlen = [int(np.asarray(V_len)[b, 0]) for b in range(B)]

    phases = []
    for b in range(B):
        Qp = _ceil_div(qlen[b], 32) * 32   # q only needs 32-elem alignment
        if Qp == 0:
            continue  # whole batch output is zero
        if vlen[b] > 0:
            NK, scale = _ceil_div(vlen[b], 128), SCALE
        else:
            # all keys masked -> reference softmax degenerates to uniform
            # over all T keys; exp(0*S + 0) = 1 reproduces it exactly.
            NK, scale = T // 128, 0.0
        phases.append(dict(b=b, NK=NK, Qp=Qp, Kp=NK * 128, scale=scale))
    phases.sort(key=lambda ph: -ph["Qp"])  # big phase first (filler donor)

    out = np.zeros((B, T, H * DH), dtype=np.float32)
    if not phases:
        return out

    nc = _build_program(phases)

    # per-phase data shared by all cores
    shared = {}
    for ph in phases:
        b, s, Qp, Kp, NK = ph["b"], str(ph["b"]), ph["Qp"], ph["Kp"], ph["NK"]
        kbias = np.where(np.arange(Kp) < vlen[b], 0.0,
                         -NEG_BIG if vlen[b] > 0 else 0.0)
        kbias = np.ascontiguousarray(
            kbias.astype(np.float32).reshape(NK, 128).T)        # [128, NK]
        d = {"kb" + s: kbias}
        for key, X, Pq in (("xq", Q_seq[b], Qp), ("xk", K_seq[b], Kp),
                           ("xv", V_seq[b], Kp)):
            full = _prep_xT(X, Pq)                              # [128, KCH, Pq]
            for ci, (c0, n) in enumerate(_chunks(Pq)):
                d[f"{key}{s}c{ci}"] = np.ascontiguousarray(
                    full[:, :, c0:c0 + n])
        shared[s] = d

    in_maps = []
    for c in range(N_CORES):
        m = {}
        for ph in phases:
            m.update(shared[str(ph["b"])])
        m["wq"] = _prep_w(WQ, c)
        m["wk"] = _prep_w(WK, c)
        m["wv"] = _prep_w(WV, c)
        in_maps.append(m)

    trace = bool(os.environ.get("BASS_TRACE"))
    if trace:
        _ensure_ntff_hook()
    res = run_bass_kernel_spmd(nc, in_maps, list(range(N_CORES)), trace=trace)
    LAST_EXEC_NS = res.exec_time_ns

    for c in range(N_CORES):
        r = res.results[c]
        for ph in phases:
            b, s, ql = ph["b"], str(ph["b"]), qlen[ph["b"]]
            o = np.asarray(r["out" + s]).astype(np.float32)  # [64, 2, Qp]
            for h in (0, 1):
                head = 2 * c + h
                out[b, :ql, head * DH:(head + 1) * DH] = o[:, h, :ql].T
    return out


# revision 18
# speedup vs baseline: 2.4002x; 1.0587x over previous
"""Trainium2 Bass kernel: masked multi-head attention, sharded across 8 NeuronCores.

Problem shapes (hardcoded): B=2, T=2048, D=1024, H=16 heads, dh=64.

Sharding: one SPMD program with two phases (one per batch element). In each
phase every core handles 2 of the 16 heads (core c -> heads 2c, 2c+1), so the
16 heads of each batch are spread over all 8 cores. This load-balances the
data-dependent work (Q_len/V_len trim the q/k tile counts per batch).

v2 changes vs the fp32 baseline:
  - bf16 inputs/weights/intermediates: matmuls run at 1 cycle/row instead of
    fp32's 4 (fp32 lowers to 2 half-speed passes on TRN2), DMA bytes halve.
  - The two heads' S^T matmuls (K=64 each) are row-tiled to disjoint PE
    quadrants (tile_position (0,0)/(64,0)) so they execute concurrently.
  - exp() for both heads merged into one ACT instruction over a 2-bank PSUM
    tile [128, 2, n] (ACT is the #2 engine; fewer/larger instrs).
  - Epilogue: numerator copied once (DVE), softmax denominator row pulled out
    of PSUM by a tiny DMA, reciprocal_approx_fast on DVE (the old
    single-lane RECIPROCAL was 2.2us/chunk), broadcast over partitions with a
    K=1 f32r matmul, one fused multiply per head.
  - Query-length masking moved to the host gather (rows >= Q_len are simply
    not copied out; the output buffer is pre-zeroed) - no qmask work on HW.
  - The second batch's projections are emitted as filler units inside the
    first batch's ACT-paced attention ladder to keep the PE busy.
"""

import math
import os
from collections import deque
from contextlib import ExitStack

import numpy as np
import ml_dtypes

import concourse.bacc as bacc
import concourse.mybir as mybir
import concourse.tile as tile
from concourse.bass_utils import run_bass_kernel_spmd

F32 = mybir.dt.float32
F32R = mybir.dt.float32r
BF16 = mybir.dt.bfloat16
EXP = mybir.ActivationFunctionType.Exp
BNP = ml_dtypes.bfloat16

B, T, D, H, DH = 2, 2048, 1024, 16, 64
N_CORES = 8
KCH = D // 128          # 8 contraction chunks of the model dim
NEG_BIG = 1.0e12
SCALE = 1.0 / math.sqrt(DH)

LAST_EXEC_NS = None     # filled when BASS_TRACE=1


def _ensure_ntff_hook():
    """run_bass_kernel_spmd(trace=True) imports antenv.axon_hooks, which some
    containers lack; synthesize it (backed by libaxon_pjrt's NRT profiling)
    so tracing degrades gracefully instead of crashing."""
    import sys
    import types
    try:
        import antenv.axon_hooks  # noqa: F401
        return
    except ImportError:
        pass
    try:
        import antenv
        from trn_agent_boot.trn_boot import _ntff_profile_via_ctypes
        hook = _ntff_profile_via_ctypes("/opt/axon/libaxon_pjrt.so")
    except Exception:
        antenv = None
        hook = None
    try:
        m = types.ModuleType("antenv.axon_hooks")
        m._hook = hook
        m.set_axon_ntff_profile_hook = lambda h: setattr(m, "_hook", h)
        m.get_axon_ntff_profile_hook = lambda: m._hook
        sys.modules["antenv.axon_hooks"] = m
        if antenv is not None:
            antenv.axon_hooks = m
    except Exception:
        pass


def _ceil_div(a, b):
    return -(-a // b)


def _chunks(total, w=512):
    out = []
    c = 0
    while c < total:
        out.append((c, min(w, total - c)))
        c += w
    return out


class _Emitter:
    def __init__(self, nc, P, wts):
        self.nc = nc
        self.P = P
        self.wts = wts

    # ---------- projection units (each returns nothing, emits instrs) ------

    def kproj_chunk(self, ph, ci):
        """Project keys chunk ci: kc[:, c0:c0+n] = (WK.T @ K_seq.T) slice."""
        nc, P = self.nc, self.P
        c0, n = ph["kch"][ci]
        xt = ph["xk_tiles"][ci]
        ps = P["pp"].tile([128, 512], F32, tag="pp", name="kps")
        for k in range(KCH):
            nc.tensor.matmul(ps[:, :n], lhsT=self.wts["wk"][:, k, :],
                             rhs=xt[:, k, :n],
                             start=(k == 0), stop=(k == KCH - 1),
                             skip_group_check=True)
        nc.vector.tensor_copy(ph["kc"][:, c0:c0 + n], ps[:, :n])

    def vproj_tile(self, ph, m):
        """Project value tokens [m*128,(m+1)*128) into va[:, m, :, 0:64]."""
        nc, P = self.nc, self.P
        ci, r = divmod(m * 128, 512)
        c0, cn = ph["vch"][ci]
        xt = ph["xv_tiles"][ci]
        ps = P["pp"].tile([128, 512], F32, tag="pp", name="vps")
        for k in range(KCH):
            nc.tensor.matmul(ps[:, 0:128], lhsT=xt[:, k, r:r + 128],
                             rhs=self.wts["wv"][:, k, :],
                             start=(k == 0), stop=(k == KCH - 1),
                             skip_group_check=True)
        nc.vector.tensor_copy(
            ph["va"][:, m, :, 0:64],
            ps[:, 0:128].rearrange("p (g d) -> p g d", g=2))

    def qproj_chunk(self, ph, ci):
        """Project queries chunk ci into the qc ring; returns the tile."""
        nc, P = self.nc, self.P
        c0, n = ph["qch"][ci]
        xt = ph["xq_tiles"][ci]
        ps = P["pp"].tile([128, 512], F32, tag="pp", name="qps")
        for k in range(KCH):
            nc.tensor.matmul(ps[:, :n], lhsT=self.wts["wq"][:, k, :],
                             rhs=xt[:, k, :n],
                             start=(k == 0), stop=(k == KCH - 1),
                             skip_group_check=True)
        qc = P["qc"].tile([128, 512], BF16, tag="qc" + str(ph["b"]),
                          name="qc", bufs=3)
        nc.vector.tensor_copy(qc[:, :n], ps[:, :n])
        ph["qcs"][ci] = qc
        return qc

    # ---------- attention ladder ------------------------------------------

    def ladder(self, ph, ci, due, anytime):
        """S/exp/PV software pipeline for q chunk ci.

        `due`: deque of (deadline_step, closure) in non-decreasing deadline
        order — every unit whose deadline has arrived is emitted that step
        (these carry dataflow deadlines, e.g. vproj(kt) before PV(kt)).
        `anytime`: deque of independent filler closures; at most one is
        popped per step, only on steps with no due unit (keeps PE work per
        step under the ACT exp cadence)."""
        nc, P = self.nc, self.P
        c0, n = ph["qch"][ci]
        NK = ph["NK"]
        qc = ph["qcs"].pop(ci)
        kb = ph["kb_tile"]
        kc, va = ph["kc"], ph["va"]
        scale = ph["scale"]

        otd = P["ot"].tile([65, 2, 512], F32, tag="ot", name="otd")

        def emit_s(kt):
            sps = P["sp"].tile([128, 2, 512], F32, tag="sp", name="sps")
            for h in (0, 1):
                nc.tensor.matmul(
                    sps[:, h, :n],
                    lhsT=kc[h * 64:(h + 1) * 64, kt * 128:(kt + 1) * 128],
                    rhs=qc[h * 64:(h + 1) * 64, :n],
                    start=True, stop=True,
                    tile_position=(h * 64, 0),
                    skip_group_check=True)
            e = P["e"].tile([128, 2, 512], BF16, tag="e", name="e", bufs=3)
            nc.scalar.activation(e[:, :, :n], sps[:, :, :n], EXP,
                                 bias=kb[:, kt:kt + 1], scale=scale)
            return e

        ep = emit_s(0)
        for kt in range(NK):
            ec = ep
            if kt + 1 < NK:
                ep = emit_s(kt + 1)
            popped = False
            while due and due[0][0] <= kt:
                due.popleft()[1]()
                popped = True
            if not popped and anytime:
                anytime.popleft()()
            for h in (0, 1):
                nc.tensor.matmul(otd[:, h, :n], lhsT=va[:, kt, h, :],
                                 rhs=ec[:, h, :n],
                                 start=(kt == 0), stop=(kt == NK - 1),
                                 skip_group_check=True)
        return otd

    def epilogue(self, ph, ci, otd):
        """Normalize otd -> OTs[:, :, c0:c0+n] (no qmask: host trims)."""
        nc, P = self.nc, self.P
        c0, n = ph["qch"][ci]
        ou = P["ou"].tile([64, 2, 512], BF16, tag="ou", name="ou", bufs=2)
        nc.vector.tensor_copy(ou[:, :, :n], otd[0:64, :, :n])
        drow = P["rows"].tile([65, 2, 512], BF16, tag="drow", name="drow",
                              bufs=2)
        nc.vector.tensor_copy(drow[64:65, :, :n], otd[64:65, :, :n])
        rsb = P["rows"].tile([64, 2, 512], F32, tag="rsb", name="rsb",
                             bufs=2)
        for h in (0, 1):
            # broadcast d over 64 partitions (K=1 bf16 matmul), then
            # reciprocal on the [64, n] block (DVE cost is free-size-based,
            # so this is no dearer than a single-partition reciprocal).
            dps = P["pp"].tile([128, 512], F32, tag="pp", name="dps")
            nc.tensor.matmul(dps[0:64, :n],
                             lhsT=P["onesr"][64:65, 0:64],
                             rhs=drow[64:65, h, :n],
                             start=True, stop=True, skip_group_check=True)
            nc.vector.reciprocal_approx_fast(rsb[:, h, :n], dps[0:64, :n])
            nc.vector.tensor_mul(ph["OTs"][:, h, c0:c0 + n],
                                 ou[:, h, :n], rsb[:, h, :n])


def _chunk0_due(em, ph):
    """Deadline units for the first q chunk's ladder: the phase's remaining
    k-proj chunks and all v-proj tiles, interleaved in the exact order their
    DMA chunks arrive, plus qproj(1). Deadlines: vproj(m) before PV(m)
    (popped a step early so the DVE copy hides), kproj(ci) before S(4ci)
    which is emitted at step 4ci-1, qproj(1) a few steps before chunk end."""
    NK = ph["NK"]
    due = []
    for m in range(NK):
        if m >= 1 and m % 4 == 0:
            due.append((m - 2, lambda ci=m // 4: em.kproj_chunk(ph, ci)))
        due.append((max(0, m - 1), lambda m=m: em.vproj_tile(ph, m)))
    if len(ph["qch"]) > 1:
        d = max(0, NK - 3)
        pos = next((i for i, u in enumerate(due) if u[0] > d), len(due))
        due.insert(pos, (d, lambda: em.qproj_chunk(ph, 1)))
    return deque(due)


def _phase_units(em, ph):
    """Independent filler closures projecting all of phase `ph`'s inputs,
    in DMA-arrival order."""
    units = []
    for ci in range(len(ph["kch"])):
        units.append(lambda ci=ci: em.kproj_chunk(ph, ci))
        for m in range(ci * 4, min((ci + 1) * 4, ph["NK"])):
            units.append(lambda m=m: em.vproj_tile(ph, m))
    units.append(lambda: em.qproj_chunk(ph, 0))
    return units


def _build_program(phases):
    nc = bacc.Bacc("TRN2", target_bir_lowering=False, debug=False,
                   num_devices=N_CORES)
    for ph in phases:
        s = str(ph["b"])
        Qp, Kp, NK = ph["Qp"], ph["Kp"], ph["NK"]
        ph["qch"] = _chunks(Qp)
        ph["kch"] = _chunks(Kp)
        ph["vch"] = ph["kch"]
        ph["qcs"] = {}
        io = {
            "kb": nc.dram_tensor("kb" + s, [128, NK], F32, kind="ExternalInput"),
            "out": nc.dram_tensor("out" + s, [64, 2, Qp], BF16, kind="ExternalOutput"),
        }
        # per-chunk input tensors: per-partition-contiguous so each DMA
        # lowers to 128 large descriptors instead of 1KB-strided fragments
        for key, chl in (("xq", ph["qch"]), ("xk", ph["kch"]), ("xv", ph["vch"])):
            for ci, (c0, n) in enumerate(chl):
                io[f"{key}c{ci}"] = nc.dram_tensor(
                    f"{key}{s}c{ci}", [128, KCH, n], BF16, kind="ExternalInput")
        ph["io"] = io

    with tile.TileContext(nc) as tc, ExitStack() as ctx:
        P = {
            "w": ctx.enter_context(tc.tile_pool(name="w", bufs=1)),
            "x": ctx.enter_context(tc.tile_pool(name="x", bufs=1)),
            "qc": ctx.enter_context(tc.tile_pool(name="qc", bufs=3)),
            "e": ctx.enter_context(tc.tile_pool(name="e", bufs=3)),
            "ou": ctx.enter_context(tc.tile_pool(name="ou", bufs=2)),
            "rows": ctx.enter_context(tc.tile_pool(name="rows", bufs=2)),
            "persist": ctx.enter_context(tc.tile_pool(name="persist", bufs=1)),
            "pp": ctx.enter_context(tc.tile_pool(name="pp", bufs=2, space="PSUM")),
            "sp": ctx.enter_context(tc.tile_pool(name="sp", bufs=2, space="PSUM")),
            "ot": ctx.enter_context(tc.tile_pool(name="ot", bufs=1, space="PSUM")),
        }
        onesr = P["w"].tile([65, 64], BF16, tag="onesr", name="onesr")
        nc.vector.memset(onesr[64:65, :], 1.0)
        P["onesr"] = onesr
        warm = P["w"].tile([1, 1], F32, tag="actwarm", name="actwarm")
        nc.vector.memset(warm[:], 0.0)
        nc.scalar.activation(warm[:], warm[:], EXP)

        # PE p-state warmup: dummy bf16 matmuls on zeroed tiles keep the PE
        # clocking up while the first input DMAs land.
        zw = P["w"].tile([128, 128], BF16, tag="zw", name="zw")
        nc.gpsimd.memset(zw[:], 0.0)
        zw2 = P["w"].tile([128, 512], BF16, tag="zw2", name="zw2")
        nc.gpsimd.memset(zw2[:], 0.0)
        for _ in range(2):
            wps = P["sp"].tile([128, 2, 512], F32, tag="sp", name="wps")
            for r in range(4):
                nc.tensor.matmul(wps[:, 0, :], lhsT=zw[:], rhs=zw2[:],
                                 start=(r == 0), stop=(r == 3),
                                 skip_group_check=True)

        # -------- weights --------
        wts = {}
        for nm in ("wk", "wq", "wv"):
            wd = nc.dram_tensor(nm, [128, KCH, 128], BF16, kind="ExternalInput")
            t = P["w"].tile([128, KCH, 128], BF16, tag=nm, name=nm)
            nc.sync.dma_start(t[:], wd[:])
            wts[nm] = t

        # -------- input staging (issue order == consumption order) --------
        A = phases[0]
        Bp = phases[1] if len(phases) > 1 else None
        for ph in phases:
            s = str(ph["b"])
            kb = P["w"].tile([128, ph["NK"]], F32, tag="kb" + s, name="kb")
            nc.sync.dma_start(kb[:], ph["io"]["kb"][:])
            ph["kb_tile"] = kb
            for key, chl in (("xq", ph["qch"]), ("xk", ph["kch"]),
                             ("xv", ph["vch"])):
                ph[f"{key}_tiles"] = [None] * len(chl)

        def stage1(ph, key, ci, sliced=False):
            s = str(ph["b"])
            n = dict(xq=ph["qch"], xk=ph["kch"], xv=ph["vch"])[key][ci][1]
            xt = P["x"].tile([128, KCH, n], BF16, tag=f"{key}{s}c{ci}",
                             name=f"{key}{s}c{ci}", bufs=1)
            src = ph["io"][f"{key}c{ci}"]
            if sliced:
                for k in range(KCH):
                    nc.sync.dma_start(xt[:, k, :], src[:, k, :])
            else:
                nc.sync.dma_start(xt[:], src[:])
            ph[f"{key}_tiles"][ci] = xt

        # order = the exact sequence the chunk-0 ladder consumes data in
        stage1(A, "xk", 0, sliced=True)
        stage1(A, "xq", 0)
        nkA, nqA = len(A["kch"]), len(A["qch"])
        stage1(A, "xv", 0)
        for ci in range(1, nkA):
            stage1(A, "xk", ci)
            if ci == nkA - 1 and nqA > 1:
                stage1(A, "xq", 1)   # qproj(1) pops just before the last kv
            stage1(A, "xv", ci)
        if nkA == 1 and nqA > 1:
            stage1(A, "xq", 1)
        for ci in range(2, nqA):
            stage1(A, "xq", ci)
        if Bp is not None:
            for ci in range(len(Bp["kch"])):
                stage1(Bp, "xk", ci)
                stage1(Bp, "xv", ci)
            for ci in range(len(Bp["qch"])):
                stage1(Bp, "xq", ci)

        # -------- persistent per-phase tiles --------
        for ph in phases:
            s = str(ph["b"])
            ph["kc"] = P["persist"].tile([128, ph["Kp"]], BF16,
                                         tag="kc" + s, name="kc" + s)
            ph["va"] = P["persist"].tile([128, ph["NK"], 2, 65], BF16,
                                         tag="va" + s, name="va" + s)
            nc.gpsimd.memset(ph["va"][:, :, :, 64:65], 1.0)
            ph["OTs"] = P["persist"].tile([64, 2, ph["Qp"]], BF16,
                                          tag="oT" + s, name="oT" + s)

        em = _Emitter(nc, P, wts)

        # -------- phase A flow --------
        em.kproj_chunk(A, 0)
        em.qproj_chunk(A, 0)
        rest = deque(_phase_units(em, Bp)) if Bp is not None else deque()
        for ci in range(nqA):
            if ci == 0:
                due = _chunk0_due(em, A)
                anytime = deque()
            else:
                due = deque()
                if ci + 1 < nqA:
                    due.append((max(0, A["NK"] - 3),
                                lambda ci=ci: em.qproj_chunk(A, ci + 1)))
                # B's fillers from chunk 2 on (their DMA lands after A's)
                anytime = rest if ci >= 2 else deque()
            otd = em.ladder(A, ci, due, anytime)
            em.epilogue(A, ci, otd)
            c0, n = A["qch"][ci]
            nc.sync.dma_start(A["io"]["out"][:, :, c0:c0 + n],
                              A["OTs"][:, :, c0:c0 + n])

        # -------- phase B flow --------
        if Bp is not None:
            while rest:
                rest.popleft()()
            if 0 not in Bp["qcs"]:
                em.qproj_chunk(Bp, 0)
            for ci in range(len(Bp["qch"])):
                otd = em.ladder(Bp, ci, deque(), deque())
                if ci + 1 < len(Bp["qch"]):
                    em.qproj_chunk(Bp, ci + 1)
                em.epilogue(Bp, ci, otd)
                c0, n = Bp["qch"][ci]
                nc.sync.dma_start(Bp["io"]["out"][:, :, c0:c0 + n],
                                  Bp["OTs"][:, :, c0:c0 + n])

    nc.compile()
    return nc


def _prep_xT(X, Pq):
    """[T, D] -> [128, KCH, Pq] bf16 with x[p, k, t] = X[t, k*128 + p]."""
    Xp = np.ascontiguousarray(X[:Pq].T)                 # [D, Pq]
    return np.ascontiguousarray(
        Xp.reshape(KCH, 128, Pq).transpose(1, 0, 2)).astype(BNP)


def _prep_w(W, c):
    """[D, H*DH] -> per-core [128, KCH, 128] bf16 slice of heads (2c, 2c+1)."""
    Ws = W[:, c * 128:(c + 1) * 128]                    # [D, 128]
    return np.ascontiguousarray(
        Ws.reshape(KCH, 128, 128).transpose(1, 0, 2)).astype(BNP)


def kernel(Q_seq, K_seq, V_seq, Q_len, V_len, WQ, WK, WV):
    global LAST_EXEC_NS
    Q_seq = np.asarray(Q_seq, dtype=np.float32)
    K_seq = np.asarray(K_seq, dtype=np.float32)
    V_seq = np.asarray(V_seq, dtype=np.float32)
    WQ = np.asarray(WQ, dtype=np.float32)
    WK = np.asarray(WK, dtype=np.float32)
    WV = np.asarray(WV, dtype=np.float32)
    qlen = [int(np.asarray(Q_len)[b, 0]) for b in range(B)]
    vlen = [int(np.asarray(V_len)[b, 0]) for b in range(B)]

    phases = []
    for b in range(B):
        Qp = _ceil_div(qlen[b], 32) * 32   # q only needs 32-elem alignment
        if Qp == 0:
            continue  # whole batch output is zero
        if vlen[b] > 0:
            NK, scale = _ceil_div(vlen[b], 128), SCALE
        else:
            # all keys masked -> reference softmax degenerates to uniform
            # over all T keys; exp(0*S + 0) = 1 reproduces it exactly.
            NK, scale = T // 128, 0.0
        phases.append(dict(b=b, NK=NK, Qp=Qp, Kp=NK * 128, scale=scale))
    phases.sort(key=lambda ph: -ph["Qp"])  # big phase first (filler donor)

    out = np.zeros((B, T, H * DH), dtype=np.float32)
    if not phases:
        return out

    nc = _build_program(phases)

    # per-phase data shared by all cores
    shared = {}
    for ph in phases:
        b, s, Qp, Kp, NK = ph["b"], str(ph["b"]), ph["Qp"], ph["Kp"], ph["NK"]
        kbias = np.where(np.arange(Kp) < vlen[b], 0.0,
                         -NEG_BIG if vlen[b] > 0 else 0.0)
        kbias = np.ascontiguousarray(
            kbias.astype(np.float32).reshape(NK, 128).T)        # [128, NK]
        d = {"kb" + s: kbias}
        for key, X, Pq in (("xq", Q_seq[b], Qp), ("xk", K_seq[b], Kp),
                           ("xv", V_seq[b], Kp)):
            full = _prep_xT(X, Pq)                              # [128, KCH, Pq]
            for ci, (c0, n) in enumerate(_chunks(Pq)):
                d[f"{key}{s}c{ci}"] = np.ascontiguousarray(
                    full[:, :, c0:c0 + n])
        shared[s] = d

    in_maps = []
    for c in range(N_CORES):
        m = {}
        for ph in phases:
            m.update(shared[str(ph["b"])])
        m["wq"] = _prep_w(WQ, c)
        m["wk"] = _prep_w(WK, c)
        m["wv"] = _prep_w(WV, c)
        in_maps.append(m)

    trace = bool(os.environ.get("BASS_TRACE"))
    if trace:
        _ensure_ntff_hook()
    res = run_bass_kernel_spmd(nc, in_maps, list(range(N_CORES)), trace=trace)
    LAST_EXEC_NS = res.exec_time_ns

    for c in range(N_CORES):
        r = res.results[c]
        for ph in phases:
            b, s, ql = ph["b"], str(ph["b"]), qlen[ph["b"]]
            o = np.asarray(r["out" + s]).astype(np.float32)  # [64, 2, Qp]
            for h in (0, 1):
                head = 2 * c + h
                out[b, :ql, head * DH:(head + 1) * DH] = o[:, h, :ql].T
    return out
